# revision 2
# baseline (speedup 1.0000x reference)
"""LocallyConnected2d (B=8, C_in=32, 48x48, C_out=32, 3x3, pad 1) on 8 trn2 cores.

Strategy: shard the spatial-location axis L = H*W across cores (6 image rows
each). Per location l the op is an (8x288)@(288x32) GEMM with location-unique
weights; weight streaming (85 MB total) dominates -> memory-bound.

Device mapping per core:
  - x halo slice lives in SBUF replicated 3x with kw column shifts, laid out
    [p=(kw*32+c), (row, col, b)], so the im2col patch for any location is a
    plain strided AP slice (no patch materialization).
  - Contraction (d=288) is split into 3 kh-rounds of K=96=(3 kw x 32 c),
    PSUM-accumulated. Round 0 carries a 97th row: ones in x, transposed bias
    in W, folding the bias add into the matmul.
  - Per location: stationary = x-view [K,8(b)] (8-column LDW, cheap),
    moving = W slice [K,32(o)], out = PSUM [8(b),32(o)].
  - W streamed in 2-image-row tiles (~1.2 MB DMAs) alternating between the
    two HWDGE rings (sync/scalar) for overlap; output kept in (r,q,o) layout
    so PSUM->SBUF copies are contiguous, final NCHW transpose done on host.
"""

import numpy as np

import concourse.bacc as bacc
import concourse.tile as tile
from concourse import mybir
from concourse.bass_utils import run_bass_kernel_spmd

B, C_IN, H, W = 8, 32, 48, 48
C_OUT = 32
N_CORES = 8
RP = H // N_CORES  # rows per core (6)
LP = RP * W  # locations per core (288)
RG = 2  # image rows per W tile
F32 = mybir.dt.float32

_nc = None


def _build():
    nc = bacc.Bacc(
        "TRN2", target_bir_lowering=False, debug=False, num_devices=N_CORES
    )
    xh = nc.dram_tensor("xh", [C_IN, RP + 2, W + 2, B], F32, kind="ExternalInput")
    w = nc.dram_tensor("w", [C_IN * 9, LP, C_OUT], F32, kind="ExternalInput")
    bt = nc.dram_tensor("bt", [LP, C_OUT], F32, kind="ExternalInput")
    # (b, r, q, o) layout so device-side stores are contiguous; host transposes.
    out = nc.dram_tensor("out", [B, RP, W, C_OUT], F32, kind="ExternalOutput")

    # w rows are d = c*9 + kh*3 + kw; expose (kh, kw, c) so one DMA per
    # (kh, row-group) lands as SBUF partitions p = kw*32 + c.
    wr = w.rearrange("(c kh kw) l o -> kh kw c l o", c=C_IN, kh=3, kw=3)

    dma_engines = None

    with tile.TileContext(nc) as tc:
        with (
            tc.tile_pool(name="xpool", bufs=1) as xpool,
            tc.tile_pool(name="wpool", bufs=6) as wpool,
            tc.tile_pool(name="opool", bufs=1) as opool,
            tc.tile_pool(name="pspool", bufs=8, space="PSUM") as pspool,
        ):
            dma_engines = [nc.sync, nc.scalar]

            x3 = xpool.tile([128, (RP + 2) * W * B], F32)
            for kw in range(3):
                dma_engines[kw % 2].dma_start(
                    x3[kw * 32 : (kw + 1) * 32, :], xh[:, :, kw : kw + W, :]
                )
            nc.vector.memset(x3[96:97, :], 1.0)

            out_sb = opool.tile([B, RP * W * C_OUT], F32)

            ndma = 0
            for g in range(RP // RG):
                wts = []
                for kh in range(3):
                    wt = wpool.tile([128, RG * W * C_OUT], F32, tag="wt")
                    dma_engines[ndma % 2].dma_start(
                        wt[0:96, :],
                        wr[kh, :, :, g * RG * W : (g + 1) * RG * W, :],
                    )
                    ndma += 1
                    if kh == 0:
                        dma_engines[ndma % 2].dma_start(
                            wt[96:97, :], bt[g * RG * W : (g + 1) * RG * W, :]
                        )
                        ndma += 1
                    wts.append(wt)
                for rl in range(g * RG, (g + 1) * RG):
                    for qg in range(W // 16):
                        ps = pspool.tile([B, 512], F32)
                        for qq in range(16):
                            q = qg * 16 + qq
                            ll = (rl - g * RG) * W + q  # loc within W tile
                            for kh in range(3):
                                kd = 97 if kh == 0 else 96
                                off = ((rl + kh) * W + q) * B
                                nc.tensor.matmul(
                                    ps[0:B, qq * 32 : (qq + 1) * 32],
                                    x3[0:kd, off : off + B],
                                    wts[kh][0:kd, ll * C_OUT : (ll + 1) * C_OUT],
                                    start=(kh == 0),
                                    stop=(kh == 2),
                                )
                        base = (rl * W + qg * 16) * C_OUT
                        nc.vector.tensor_copy(
                            out_sb[0:B, base : base + 512], ps[0:B, :]
                        )
            nc.sync.dma_start(out[:, :, :, :], out_sb[0:B, :])
    nc.compile()
    return nc


def _shard(inputs):
    x = np.asarray(inputs["x"], np.float32)
    weight = np.asarray(inputs["weight"], np.float32)[0]
    bias = np.asarray(inputs["bias"], np.float32)[0]
    xp = np.pad(x, ((0, 0), (0, 0), (1, 1), (1, 1)))
    bias_t = np.ascontiguousarray(bias.reshape(C_OUT, H * W).T)
    in_maps = []
    for k in range(N_CORES):
        r0 = RP * k
        in_maps.append(
            {
                "xh": np.ascontiguousarray(
                    xp[:, :, r0 : r0 + RP + 2, :].transpose(1, 2, 3, 0)
                ),
                "w": np.ascontiguousarray(weight[:, LP * k : LP * (k + 1), :]),
                "bt": np.ascontiguousarray(bias_t[LP * k : LP * (k + 1), :]),
            }
        )
    return in_maps


def _get_nc():
    global _nc
    if _nc is None:
        _nc = _build()
    return _nc


def _gather(results):
    # per-core out is (B, RP, W, C_OUT); concat rows, then to NCHW
    full = np.concatenate([results[k]["out"] for k in range(N_CORES)], axis=1)
    return np.ascontiguousarray(full.transpose(0, 3, 1, 2))


def kernel(**inputs):
    nc = _get_nc()
    res = run_bass_kernel_spmd(nc, _shard(inputs), list(range(N_CORES)))
    return _gather(res.results)
